# revision 7
# baseline (speedup 1.0000x reference)
"""Multi-head attention with random-synthesizer blend + mask, on 8 Trainium2
NeuronCores.

Sharding: data-parallel over batch (B=8 -> one batch element per core).

Per-core algorithm (S=1024, D=1024, H=16, HD=64), all layouts [partition, free]:
  - x_T = transpose(x) for x in {query,key,value} via PE transposes.
  - q_T = c1*(Wq^T x^T + bq) in [d_out, s] layout (fp32r matmuls,
    c1 = alpha/sqrt(HD) folded into the PSUM->SBUF evacuation scale).
  - k_T likewise (scale 1), v in natural [s, d_out] layout (bf16, with
    interleaved all-ones 64-column blocks used to compute softmax sums).
  - Per (head, k-chunk): scores_T = k_T[h]^T q_T[h] (PSUM), += c2*syn_T
    (c2-scaled-identity matmul against transpose-DMA-loaded bf16 syn tiles),
    p = exp(.) via ACT, p *= mask_T (bf16, prepared on-chip by PE transpose),
    out_T[h] accumulated = [v[h] | ones]^T p over k-chunks; PSUM rows give
    both the unnormalized output and the softmax denominators.
  - Normalize with DVE reciprocal+mult (denominators moved to the matching
    partition half with a small SBUF->SBUF DMA), then out = otn^T Wo + bo'.

Host-side prep is limited to slicing/sharding, the sigmoid of the scalar
alpha parameter, a bf16 cast of the synthesizer scores (the 2-byte dtype is
what makes the hardware DMA-transpose path legal), and folding the zero-cost
bias identity bo' = bv @ Wo + bo (exact: softmax weights sum to 1, so the
v-bias shifts attention output by bv). alpha is folded into compiled
constants; the program is rebuilt if alpha changes.
"""

import math
import sys

sys.path.insert(0, "/opt/trn_rl_repo")

import ml_dtypes
import numpy as np

import concourse.tile as tile
import concourse.mybir as mybir
from concourse import bacc
from concourse.bass_utils import run_bass_kernel_spmd
from concourse.masks import make_identity

B, S, D, H = 8, 1024, 1024, 16
HD = D // H  # 64
N_CORES = 8
P = 128
SC = S // P  # 8
DC = D // P  # 8
NQ = 512
QC = S // NQ  # 2

f32 = mybir.dt.float32
f32r = mybir.dt.float32r
bf16 = mybir.dt.bfloat16
i32 = mybir.dt.int32
AF = mybir.ActivationFunctionType
OP = mybir.AluOpType

# test harness knobs (the grading entry point `kernel` leaves these alone)
TRACE = False
TRACE_TMPDIR = None
LAST_RESULTS = None

_CACHE = {}


def _emit(nc, tc, dram, c1, c2):
    xin = {"q": dram["xq"], "k": dram["xk"], "v": dram["xv"]}
    w_d = {"q": dram["wq"], "k": dram["wk"], "v": dram["wv"], "o": dram["wo"]}
    msk_d, syn_d, out_d = dram["msk"], dram["syn"], dram["out"]

    with (
        tc.tile_pool(name="pers", bufs=1) as pers,
        tc.tile_pool(name="psum", bufs=1, space="PSUM") as psum,
    ):
        # ---- constants ---------------------------------------------------
        ident1f = pers.tile([P, P], f32, tag="ident1f")
        make_identity(nc, ident1f[:])
        ident1b = pers.tile([P, P], bf16, tag="ident1b")
        make_identity(nc, ident1b[:])
        identc2 = pers.tile([P, P], bf16, tag="identc2")
        make_identity(nc, identc2[:])
        if c2 != 1.0:
            nc.vector.tensor_scalar(
                out=identc2[:], in0=identc2[:], scalar1=float(c2), scalar2=None,
                op0=OP.mult,
            )
        ones_r = pers.tile([1, P], f32, tag="ones_r")
        nc.vector.memset(ones_r[:], 1.0)
        ones_rr = pers.tile([1, P], f32r, tag="ones_rr")
        nc.vector.tensor_copy(out=ones_rr[:], in_=ones_r[:])

        bqk_sb = {}
        for nm in ("q", "k"):
            t = pers.tile([P, DC], f32, tag=f"b{nm}", name=f"b{nm}")
            nc.sync.dma_start(out=t[:], in_=dram["b" + nm].rearrange("(c p) -> p c", p=P))
            bqk_sb[nm] = t
        if c1 != 1.0:
            nc.vector.tensor_scalar(
                out=bqk_sb["q"][:], in0=bqk_sb["q"][:], scalar1=float(c1),
                scalar2=None, op0=OP.mult,
            )
        # bo' = bv @ Wo + bo, prepared by the host into dram["boeff"]
        bo_sb = pers.tile([1, D], f32r, tag="bo_sb")

        # ---- persistent activations --------------------------------------
        qT = [pers.tile([P, S], f32r, tag=f"qT{i}", name=f"qT{i}") for i in range(DC)]
        kT = [pers.tile([P, S], f32r, tag=f"kT{i}", name=f"kT{i}") for i in range(DC)]

        def load_w_chunks(nm, wpool, rawpool):
            tiles = []
            for ci in range(DC):
                t0 = rawpool.tile([P, D], f32, tag="wraw", name=f"wr{nm}{ci}")
                nc.sync.dma_start(out=t0[:], in_=w_d[nm][ci * P:(ci + 1) * P, :])
                t = wpool.tile([P, D], f32r, tag=f"w{ci}", name=f"w{nm}{ci}")
                nc.vector.tensor_copy(out=t[:], in_=t0[:])
                tiles.append(t)
            return tiles

        # ================= phase 1: projections ==========================
        with tc.tile_pool(name="prolog", bufs=1) as pro:
            b0 = pro.tile([1, D], f32, tag="braw")
            nc.sync.dma_start(out=b0[:], in_=dram["boeff"][None, :])
            nc.vector.tensor_copy(out=bo_sb[:], in_=b0[:])

            def transpose_in(x_d, dst_tiles):
                for sc in range(SC):
                    t0 = pro.tile([P, D], f32, tag="xraw", bufs=2, name=f"xr{sc}")
                    nc.sync.dma_start(out=t0[:], in_=x_d[sc * P:(sc + 1) * P, :])
                    for di in range(DC):
                        tp = psum.tile([P, P], f32, tag="tp", bufs=2, name="tpx")
                        nc.tensor.transpose(
                            tp[:], t0[:, di * P:(di + 1) * P], ident1f[:]
                        )
                        dst = dst_tiles[di][:, sc * P:(sc + 1) * P]
                        if (sc + di) % 2:
                            nc.vector.tensor_copy(out=dst, in_=tp[:])
                        else:
                            nc.scalar.copy(out=dst, in_=tp[:])

            # q_T / k_T: [d_out, s]
            for nm, dst, scale in (("q", qT, c1), ("k", kT, 1.0)):
                wt = load_w_chunks(nm, pro, pro)
                xT = [pro.tile([P, S], f32r, tag=f"xT{i}", name=f"xT{nm}{i}")
                      for i in range(DC)]
                transpose_in(xin[nm], xT)
                for do in range(DC):
                    for sq in range(QC):
                        ps = psum.tile([P, NQ], f32, tag="mm", bufs=3, name="psp")
                        for di in range(DC):
                            nc.tensor.matmul(
                                ps[:],
                                wt[di][:, do * P:(do + 1) * P],
                                xT[di][:, sq * NQ:(sq + 1) * NQ],
                                start=(di == 0),
                                stop=(di == DC - 1),
                            )
                        nc.scalar.activation(
                            out=dst[do][:, sq * NQ:(sq + 1) * NQ], in_=ps[:],
                            func=AF.Identity, bias=bqk_sb[nm][:, do:do + 1],
                            scale=float(scale),
                        )

            # v natural [s, d_out] into interleaved [v|ones] blocks (bf16)
            v_sb = [pers.tile([P, H * P], bf16, tag=f"v{i}", name=f"v{i}")
                    for i in range(SC)]
            wt = load_w_chunks("v", pro, pro)
            xT = [pro.tile([P, S], f32r, tag=f"xT{i}", name=f"xTv{i}")
                  for i in range(DC)]
            transpose_in(xin["v"], xT)
            for sc in range(SC):
                nc.vector.memset(v_sb[sc][:], 1.0)
            for sc in range(SC):
                for dq in range(QC):
                    ps = psum.tile([P, NQ], f32, tag="mm", bufs=3, name="psv")
                    for di in range(DC):
                        nc.tensor.matmul(
                            ps[:],
                            xT[di][:, sc * P:(sc + 1) * P],
                            wt[di][:, dq * NQ:(dq + 1) * NQ],
                            start=(di == 0),
                            stop=(di == DC - 1),
                        )
                    for j in range(NQ // HD):
                        h = dq * (NQ // HD) + j
                        off = h * P + (HD if h % 2 else 0)
                        nc.scalar.copy(
                            out=v_sb[sc][:, off:off + HD],
                            in_=ps[:, j * HD:(j + 1) * HD],
                        )

        # ================= mask prep =====================================
        otnp_cm = tc.tile_pool(name="otnp", bufs=1)
        otnp = otnp_cm.__enter__()
        mtp_cm = tc.tile_pool(name="mtp", bufs=1)
        mtp = mtp_cm.__enter__()
        maskT = [mtp.tile([P, S], bf16, tag=f"mT{i}", name=f"mT{i}")
                 for i in range(SC)]
        with tc.tile_pool(name="mpool", bufs=2) as mp:
            for qb in range(SC):
                m0 = mp.tile([P, S], i32, tag="mraw", name=f"mr{qb}")
                nc.sync.dma_start(out=m0[:], in_=msk_d[qb * P:(qb + 1) * P, :])
                mb = mp.tile([P, S], bf16, tag="mbf", name=f"mb{qb}")
                nc.vector.tensor_copy(out=mb[:], in_=m0[:])
                for kb in range(SC):
                    tp = psum.tile([P, P], bf16, tag="tp", bufs=2, name="tpm")
                    nc.tensor.transpose(
                        tp[:], mb[:, kb * P:(kb + 1) * P], ident1b[:]
                    )
                    if (qb + kb) % 2:
                        nc.vector.tensor_copy(
                            out=maskT[kb][:, qb * P:(qb + 1) * P], in_=tp[:]
                        )
                    else:
                        nc.scalar.copy(
                            out=maskT[kb][:, qb * P:(qb + 1) * P], in_=tp[:]
                        )

        # ================= phase 2: attention ============================
        otn = [otnp.tile([P, S], f32r, tag=f"otn{i}", name=f"otn{i}")
               for i in range(DC)]
        with tc.tile_pool(name="attn", bufs=1) as ap:
            for h in range(H):
                hp, hodd = h // 2, h % 2
                pav = [psum.tile([P, NQ], f32, tag="av", bufs=3, name=f"pav{h}_{i}")
                       for i in range(QC)]
                for kc in range(SC):
                    syn_t = ap.tile([P, S], bf16, tag="synT", bufs=4,
                                    name=f"sy{h}_{kc}")
                    nc.sync.dma_start_transpose(
                        out=syn_t[:], in_=syn_d[h, :, kc * P:(kc + 1) * P]
                    )
                    for sq in range(QC):
                        ps = psum.tile([P, NQ], f32, tag="mm", bufs=3, name="pss")
                        nc.tensor.matmul(
                            ps[:],
                            kT[hp][hodd * HD:(hodd + 1) * HD, kc * P:(kc + 1) * P],
                            qT[hp][hodd * HD:(hodd + 1) * HD, sq * NQ:(sq + 1) * NQ],
                            start=True, stop=False,
                        )
                        nc.tensor.matmul(
                            ps[:], identc2[:], syn_t[:, sq * NQ:(sq + 1) * NQ],
                            start=False, stop=True,
                        )
                        p = ap.tile([P, NQ], bf16, tag="p", bufs=3, name="p")
                        nc.scalar.activation(out=p[:], in_=ps[:], func=AF.Exp)
                        pm = ap.tile([P, NQ], bf16, tag="pm", bufs=3, name="pm")
                        nc.vector.tensor_tensor(
                            out=pm[:], in0=p[:],
                            in1=maskT[kc][:, sq * NQ:(sq + 1) * NQ], op=OP.mult,
                        )
                        nc.tensor.matmul(
                            pav[sq][:], v_sb[kc][:, h * P:(h + 1) * P], pm[:],
                            start=(kc == 0), stop=(kc == SC - 1),
                        )
                # normalize; out rows at [64*hodd, +64), sums on the other half
                olo, slo = HD * hodd, HD * (1 - hodd)
                for sq in range(QC):
                    stage = ap.tile([P, NQ], f32, tag="stage", bufs=2, name="stg")
                    nc.scalar.copy(
                        out=stage[slo:slo + HD, :], in_=pav[sq][slo:slo + HD, :]
                    )
                    sums2 = ap.tile([P, NQ], f32, tag="sums2", bufs=2, name="sm2")
                    nc.gpsimd.dma_start(
                        out=sums2[olo:olo + HD, :], in_=stage[slo:slo + HD, :]
                    )
                    rec = ap.tile([P, NQ], f32, tag="rec", bufs=2, name="rec")
                    nc.vector.reciprocal(
                        out=rec[olo:olo + HD, :], in_=sums2[olo:olo + HD, :]
                    )
                    nc.vector.tensor_tensor(
                        out=otn[hp][olo:olo + HD, sq * NQ:(sq + 1) * NQ],
                        in0=pav[sq][olo:olo + HD, :], in1=rec[olo:olo + HD, :],
                        op=OP.mult,
                    )

        mtp_cm.__exit__(None, None, None)

        # ================= phase 3: output projection ====================
        with tc.tile_pool(name="wo", bufs=1) as wop:
            wt = load_w_chunks("o", wop, wop)
            for sc in range(SC):
                for dq in range(QC):
                    ps = psum.tile([P, NQ], f32, tag="mm", bufs=3, name="pso")
                    for ci in range(DC):
                        nc.tensor.matmul(
                            ps[:],
                            otn[ci][:, sc * P:(sc + 1) * P],
                            wt[ci][:, dq * NQ:(dq + 1) * NQ],
                            start=(ci == 0), stop=False,
                        )
                    nc.tensor.matmul(
                        ps[:], ones_rr[:, :P], bo_sb[:, dq * NQ:(dq + 1) * NQ],
                        start=False, stop=True,
                    )
                    osb = wop.tile([P, NQ], f32, tag="osb", bufs=2, name="osb")
                    nc.scalar.copy(out=osb[:], in_=ps[:])
                    nc.sync.dma_start(
                        out=out_d[sc * P:(sc + 1) * P, dq * NQ:(dq + 1) * NQ],
                        in_=osb[:],
                    )
        otnp_cm.__exit__(None, None, None)


def _build(c1, c2):
    nc = bacc.Bacc("TRN2", debug=False)
    dram = {
        "xq": nc.declare_dram_parameter("xq", [S, D], f32, isOutput=False),
        "xk": nc.declare_dram_parameter("xk", [S, D], f32, isOutput=False),
        "xv": nc.declare_dram_parameter("xv", [S, D], f32, isOutput=False),
        "msk": nc.declare_dram_parameter("msk", [S, S], i32, isOutput=False),
        "wq": nc.declare_dram_parameter("wq", [D, D], f32, isOutput=False),
        "wk": nc.declare_dram_parameter("wk", [D, D], f32, isOutput=False),
        "wv": nc.declare_dram_parameter("wv", [D, D], f32, isOutput=False),
        "wo": nc.declare_dram_parameter("wo", [D, D], f32, isOutput=False),
        "bq": nc.declare_dram_parameter("bq", [D], f32, isOutput=False),
        "bk": nc.declare_dram_parameter("bk", [D], f32, isOutput=False),
        "boeff": nc.declare_dram_parameter("boeff", [D], f32, isOutput=False),
        "syn": nc.declare_dram_parameter("syn", [H, S, S], bf16, isOutput=False),
        "out": nc.declare_dram_parameter("out", [S, D], f32, isOutput=True),
    }
    with tile.TileContext(nc) as tc:
        _emit(nc, tc, dram, c1, c2)
    nc.compile()
    return nc


def kernel(**inputs):
    global LAST_RESULTS
    q = np.asarray(inputs["query"], np.float32)
    k = np.asarray(inputs["key"], np.float32)
    v = np.asarray(inputs["value"], np.float32)
    msk = np.asarray(inputs["mask"], np.int32)
    ws = {nm: np.asarray(inputs["W" + nm], np.float32) for nm in "qkvo"}
    bs = {nm: np.asarray(inputs["b" + nm], np.float32) for nm in "qkvo"}
    alpha = float(1.0 / (1.0 + math.exp(-float(np.asarray(inputs["alpha_param"]).ravel()[0]))))
    c1 = alpha / math.sqrt(HD)
    c2 = 1.0 - alpha
    syn_bf = np.ascontiguousarray(
        np.asarray(inputs["syn_scores"])[:, :S, :S]).astype(ml_dtypes.bfloat16)
    boeff = (bs["v"].astype(np.float64) @ ws["o"].astype(np.float64)
             + bs["o"]).astype(np.float32)

    key_ = (round(c1, 12), round(c2, 12))
    if key_ not in _CACHE:
        _CACHE[key_] = _build(c1, c2)
    nc = _CACHE[key_]

    in_maps = []
    for b in range(B):
        in_maps.append({
            "xq": np.ascontiguousarray(q[b]),
            "xk": np.ascontiguousarray(k[b]),
            "xv": np.ascontiguousarray(v[b]),
            "msk": np.ascontiguousarray(msk[b]),
            "wq": ws["q"], "wk": ws["k"], "wv": ws["v"], "wo": ws["o"],
            "bq": bs["q"], "bk": bs["k"], "boeff": boeff,
            "syn": syn_bf,
        })

    kwargs = {}
    if TRACE:
        kwargs["trace"] = True
        if TRACE_TMPDIR:
            kwargs["tmpdir"] = TRACE_TMPDIR
    res = run_bass_kernel_spmd(nc, in_maps, core_ids=list(range(N_CORES)), **kwargs)
    LAST_RESULTS = res
    return np.stack([res.results[b]["out"] for b in range(B)], axis=0)


# revision 10
# speedup vs baseline: 1.0099x; 1.0099x over previous
"""Multi-head attention with random-synthesizer blend + mask, on 8 Trainium2
NeuronCores.

Sharding: data-parallel over batch (B=8 -> one batch element per core).

Per-core algorithm (S=1024, D=1024, H=16, HD=64), all layouts [partition, free]:
  - x_T = transpose(x) for x in {query,key,value} via PE transposes.
  - q_T = c1*(Wq^T x^T + bq) in [d_out, s] layout (fp32r matmuls,
    c1 = alpha/sqrt(HD) folded into the PSUM->SBUF evacuation scale).
  - k_T likewise (scale 1), v in natural [s, d_out] layout (bf16, with
    interleaved all-ones 64-column blocks used to compute softmax sums).
  - Per (head, k-chunk): scores_T = k_T[h]^T q_T[h] (PSUM), += c2*syn_T
    (c2-scaled-identity matmul against transpose-DMA-loaded bf16 syn tiles),
    p = exp(.) via ACT, p *= mask_T (bf16, prepared on-chip by PE transpose),
    out_T[h] accumulated = [v[h] | ones]^T p over k-chunks; PSUM rows give
    both the unnormalized output and the softmax denominators.
  - Normalize with DVE reciprocal+mult (denominators moved to the matching
    partition half with a small SBUF->SBUF DMA), then out = otn^T Wo + bo'.

Host-side prep is limited to slicing/sharding, the sigmoid of the scalar
alpha parameter, a bf16 cast of the synthesizer scores (the 2-byte dtype is
what makes the hardware DMA-transpose path legal), and folding the zero-cost
bias identity bo' = bv @ Wo + bo (exact: softmax weights sum to 1, so the
v-bias shifts attention output by bv). alpha is folded into compiled
constants; the program is rebuilt if alpha changes.
"""

import math
import sys

sys.path.insert(0, "/opt/trn_rl_repo")

import ml_dtypes
import numpy as np

import concourse.tile as tile
import concourse.mybir as mybir
from concourse import bacc
from concourse.bass_utils import run_bass_kernel_spmd
from concourse.masks import make_identity

B, S, D, H = 8, 1024, 1024, 16
HD = D // H  # 64
N_CORES = 8
P = 128
SC = S // P  # 8
DC = D // P  # 8
NQ = 512
QC = S // NQ  # 2

f32 = mybir.dt.float32
f32r = mybir.dt.float32r
bf16 = mybir.dt.bfloat16
i32 = mybir.dt.int32
AF = mybir.ActivationFunctionType
OP = mybir.AluOpType

# test harness knobs (the grading entry point `kernel` leaves these alone)
TRACE = False
TRACE_TMPDIR = None
LAST_RESULTS = None

_CACHE = {}


def _emit(nc, tc, dram, c1, c2):
    xin = {"q": dram["xq"], "k": dram["xk"], "v": dram["xv"]}
    w_d = {"q": dram["wq"], "k": dram["wk"], "v": dram["wv"], "o": dram["wo"]}
    msk_d, syn_d, out_d = dram["msk"], dram["syn"], dram["out"]

    with (
        tc.tile_pool(name="pers", bufs=1) as pers,
        tc.tile_pool(name="psum", bufs=1, space="PSUM") as psum,
    ):
        # ---- constants ---------------------------------------------------
        ident1f = pers.tile([P, P], f32, tag="ident1f")
        make_identity(nc, ident1f[:])
        ident1b = pers.tile([P, P], bf16, tag="ident1b")
        make_identity(nc, ident1b[:])
        identc2 = pers.tile([P, P], bf16, tag="identc2")
        make_identity(nc, identc2[:])
        if c2 != 1.0:
            nc.vector.tensor_scalar(
                out=identc2[:], in0=identc2[:], scalar1=float(c2), scalar2=None,
                op0=OP.mult,
            )
        ones_r = pers.tile([1, P], f32, tag="ones_r")
        nc.vector.memset(ones_r[:], 1.0)
        ones_rr = pers.tile([1, P], f32r, tag="ones_rr")
        nc.vector.tensor_copy(out=ones_rr[:], in_=ones_r[:])

        bqk_sb = {}
        for nm in ("q", "k"):
            t = pers.tile([P, DC], f32, tag=f"b{nm}", name=f"b{nm}")
            nc.sync.dma_start(out=t[:], in_=dram["b" + nm].rearrange("(c p) -> p c", p=P))
            bqk_sb[nm] = t
        if c1 != 1.0:
            nc.vector.tensor_scalar(
                out=bqk_sb["q"][:], in0=bqk_sb["q"][:], scalar1=float(c1),
                scalar2=None, op0=OP.mult,
            )
        # bo' = bv @ Wo + bo, prepared by the host into dram["boeff"]
        bo_sb = pers.tile([1, D], f32r, tag="bo_sb")

        # ---- persistent activations --------------------------------------
        qT = [pers.tile([P, S], f32r, tag=f"qT{i}", name=f"qT{i}") for i in range(DC)]
        kT = [pers.tile([P, S], f32r, tag=f"kT{i}", name=f"kT{i}") for i in range(DC)]

        def load_w_chunks(nm, wpool, rawpool):
            tiles = []
            for ci in range(DC):
                t0 = rawpool.tile([P, D], f32, tag="wraw", name=f"wr{nm}{ci}")
                nc.sync.dma_start(out=t0[:], in_=w_d[nm][ci * P:(ci + 1) * P, :])
                t = wpool.tile([P, D], f32r, tag=f"w{ci}", name=f"w{nm}{ci}")
                nc.vector.tensor_copy(out=t[:], in_=t0[:])
                tiles.append(t)
            return tiles

        # ================= phase 1: projections ==========================
        pstp_cm = tc.tile_pool(name="pstp", bufs=1, space="PSUM")
        pstp = pstp_cm.__enter__()
        with tc.tile_pool(name="prolog", bufs=1) as pro:
            b0 = pro.tile([1, D], f32, tag="braw")
            nc.sync.dma_start(out=b0[:], in_=dram["boeff"][None, :])
            nc.vector.tensor_copy(out=bo_sb[:], in_=b0[:])

            def transpose_in(x_d, dst_tiles):
                for sc in range(SC):
                    t0 = pro.tile([P, D], f32, tag="xraw", bufs=2, name=f"xr{sc}")
                    nc.sync.dma_start(out=t0[:], in_=x_d[sc * P:(sc + 1) * P, :])
                    for di in range(DC):
                        tp = pstp.tile([P, P], f32, tag="tp", bufs=2, name="tpx")
                        nc.tensor.transpose(
                            tp[:], t0[:, di * P:(di + 1) * P], ident1f[:]
                        )
                        dst = dst_tiles[di][:, sc * P:(sc + 1) * P]
                        if (sc + di) % 2:
                            nc.vector.tensor_copy(out=dst, in_=tp[:])
                        else:
                            nc.scalar.copy(out=dst, in_=tp[:])

            # q_T / k_T: [d_out, s]
            for nm, dst, scale in (("q", qT, c1), ("k", kT, 1.0)):
                wt = load_w_chunks(nm, pro, pro)
                xT = [pro.tile([P, S], f32r, tag=f"xT{i}", name=f"xT{nm}{i}")
                      for i in range(DC)]
                transpose_in(xin[nm], xT)
                for do in range(DC):
                    for sq in range(QC):
                        ps = psum.tile([P, NQ], f32, tag="mm", bufs=3, name="psp")
                        for di in range(DC):
                            nc.tensor.matmul(
                                ps[:],
                                wt[di][:, do * P:(do + 1) * P],
                                xT[di][:, sq * NQ:(sq + 1) * NQ],
                                start=(di == 0),
                                stop=(di == DC - 1),
                            )
                        nc.scalar.activation(
                            out=dst[do][:, sq * NQ:(sq + 1) * NQ], in_=ps[:],
                            func=AF.Identity, bias=bqk_sb[nm][:, do:do + 1],
                            scale=float(scale),
                        )

            # v natural [s, d_out] into interleaved [v|ones] blocks (bf16)
            v_sb = [pers.tile([P, H * P], bf16, tag=f"v{i}", name=f"v{i}")
                    for i in range(SC)]
            wt = load_w_chunks("v", pro, pro)
            xT = [pro.tile([P, S], f32r, tag=f"xT{i}", name=f"xTv{i}")
                  for i in range(DC)]
            transpose_in(xin["v"], xT)
            for sc in range(SC):
                nc.vector.memset(v_sb[sc][:], 1.0)
            for sc in range(SC):
                for dq in range(QC):
                    ps = psum.tile([P, NQ], f32, tag="mm", bufs=3, name="psv")
                    for di in range(DC):
                        nc.tensor.matmul(
                            ps[:],
                            xT[di][:, sc * P:(sc + 1) * P],
                            wt[di][:, dq * NQ:(dq + 1) * NQ],
                            start=(di == 0),
                            stop=(di == DC - 1),
                        )
                    for j in range(NQ // HD):
                        h = dq * (NQ // HD) + j
                        off = h * P + (HD if h % 2 else 0)
                        nc.scalar.copy(
                            out=v_sb[sc][:, off:off + HD],
                            in_=ps[:, j * HD:(j + 1) * HD],
                        )

        # ================= mask prep =====================================
        otnp_cm = tc.tile_pool(name="otnp", bufs=1)
        otnp = otnp_cm.__enter__()
        mtp_cm = tc.tile_pool(name="mtp", bufs=1)
        mtp = mtp_cm.__enter__()
        maskT = [mtp.tile([P, S], bf16, tag=f"mT{i}", name=f"mT{i}")
                 for i in range(SC)]
        with tc.tile_pool(name="mpool", bufs=2) as mp:
            for qb in range(SC):
                m0 = mp.tile([P, S], i32, tag="mraw", name=f"mr{qb}")
                nc.sync.dma_start(out=m0[:], in_=msk_d[qb * P:(qb + 1) * P, :])
                mb = mp.tile([P, S], bf16, tag="mbf", name=f"mb{qb}")
                nc.vector.tensor_copy(out=mb[:], in_=m0[:])
                for kb in range(SC):
                    tp = pstp.tile([P, P], bf16, tag="tp", bufs=2, name="tpm")
                    nc.tensor.transpose(
                        tp[:], mb[:, kb * P:(kb + 1) * P], ident1b[:]
                    )
                    if (qb + kb) % 2:
                        nc.vector.tensor_copy(
                            out=maskT[kb][:, qb * P:(qb + 1) * P], in_=tp[:]
                        )
                    else:
                        nc.scalar.copy(
                            out=maskT[kb][:, qb * P:(qb + 1) * P], in_=tp[:]
                        )

        pstp_cm.__exit__(None, None, None)

        # ================= phase 2: attention ============================
        otn = [otnp.tile([P, S], f32r, tag=f"otn{i}", name=f"otn{i}")
               for i in range(DC)]
        with (
            tc.tile_pool(name="attn", bufs=1) as ap,
            tc.tile_pool(name="psav", bufs=1, space="PSUM") as psav,
        ):
            for h in range(H):
                hp, hodd = h // 2, h % 2
                pav = [psav.tile([P, NQ], f32, tag="av", bufs=4, name=f"pav{h}_{i}")
                       for i in range(QC)]
                for kc in range(SC):
                    syn_t = ap.tile([P, S], bf16, tag="synT", bufs=6,
                                    name=f"sy{h}_{kc}")
                    nc.sync.dma_start_transpose(
                        out=syn_t[:], in_=syn_d[h, :, kc * P:(kc + 1) * P]
                    )
                    for sq in range(QC):
                        ps = psum.tile([P, NQ], f32, tag="mm", bufs=3, name="pss")
                        nc.tensor.matmul(
                            ps[:],
                            kT[hp][hodd * HD:(hodd + 1) * HD, kc * P:(kc + 1) * P],
                            qT[hp][hodd * HD:(hodd + 1) * HD, sq * NQ:(sq + 1) * NQ],
                            start=True, stop=False,
                        )
                        nc.tensor.matmul(
                            ps[:], identc2[:], syn_t[:, sq * NQ:(sq + 1) * NQ],
                            start=False, stop=True,
                        )
                        p = ap.tile([P, NQ], bf16, tag="p", bufs=4, name="p")
                        nc.scalar.activation(out=p[:], in_=ps[:], func=AF.Exp)
                        pm = ap.tile([P, NQ], bf16, tag="pm", bufs=4, name="pm")
                        nc.vector.tensor_tensor(
                            out=pm[:], in0=p[:],
                            in1=maskT[kc][:, sq * NQ:(sq + 1) * NQ], op=OP.mult,
                        )
                        nc.tensor.matmul(
                            pav[sq][:], v_sb[kc][:, h * P:(h + 1) * P], pm[:],
                            start=(kc == 0), stop=(kc == SC - 1),
                        )
                # normalize; out rows at [64*hodd, +64), sums on the other half
                olo, slo = HD * hodd, HD * (1 - hodd)
                for sq in range(QC):
                    stage = ap.tile([P, NQ], f32, tag="stage", bufs=3, name="stg")
                    nc.scalar.copy(
                        out=stage[slo:slo + HD, :], in_=pav[sq][slo:slo + HD, :]
                    )
                    sums2 = ap.tile([P, NQ], f32, tag="sums2", bufs=3, name="sm2")
                    nc.gpsimd.dma_start(
                        out=sums2[olo:olo + HD, :], in_=stage[slo:slo + HD, :]
                    )
                    rec = ap.tile([P, NQ], f32, tag="rec", bufs=3, name="rec")
                    nc.vector.reciprocal(
                        out=rec[olo:olo + HD, :], in_=sums2[olo:olo + HD, :]
                    )
                    nc.vector.tensor_tensor(
                        out=otn[hp][olo:olo + HD, sq * NQ:(sq + 1) * NQ],
                        in0=pav[sq][olo:olo + HD, :], in1=rec[olo:olo + HD, :],
                        op=OP.mult,
                    )

        mtp_cm.__exit__(None, None, None)

        # ================= phase 3: output projection ====================
        with tc.tile_pool(name="wo", bufs=1) as wop:
            wt = load_w_chunks("o", wop, wop)
            for sc in range(SC):
                for dq in range(QC):
                    ps = psum.tile([P, NQ], f32, tag="mm", bufs=3, name="pso")
                    for ci in range(DC):
                        nc.tensor.matmul(
                            ps[:],
                            otn[ci][:, sc * P:(sc + 1) * P],
                            wt[ci][:, dq * NQ:(dq + 1) * NQ],
                            start=(ci == 0), stop=False,
                        )
                    nc.tensor.matmul(
                        ps[:], ones_rr[:, :P], bo_sb[:, dq * NQ:(dq + 1) * NQ],
                        start=False, stop=True,
                    )
                    osb = wop.tile([P, NQ], f32, tag="osb", bufs=2, name="osb")
                    nc.scalar.copy(out=osb[:], in_=ps[:])
                    nc.sync.dma_start(
                        out=out_d[sc * P:(sc + 1) * P, dq * NQ:(dq + 1) * NQ],
                        in_=osb[:],
                    )
        otnp_cm.__exit__(None, None, None)


def _build(c1, c2):
    nc = bacc.Bacc("TRN2", debug=False)
    dram = {
        "xq": nc.declare_dram_parameter("xq", [S, D], f32, isOutput=False),
        "xk": nc.declare_dram_parameter("xk", [S, D], f32, isOutput=False),
        "xv": nc.declare_dram_parameter("xv", [S, D], f32, isOutput=False),
        "msk": nc.declare_dram_parameter("msk", [S, S], i32, isOutput=False),
        "wq": nc.declare_dram_parameter("wq", [D, D], f32, isOutput=False),
        "wk": nc.declare_dram_parameter("wk", [D, D], f32, isOutput=False),
        "wv": nc.declare_dram_parameter("wv", [D, D], f32, isOutput=False),
        "wo": nc.declare_dram_parameter("wo", [D, D], f32, isOutput=False),
        "bq": nc.declare_dram_parameter("bq", [D], f32, isOutput=False),
        "bk": nc.declare_dram_parameter("bk", [D], f32, isOutput=False),
        "boeff": nc.declare_dram_parameter("boeff", [D], f32, isOutput=False),
        "syn": nc.declare_dram_parameter("syn", [H, S, S], bf16, isOutput=False),
        "out": nc.declare_dram_parameter("out", [S, D], f32, isOutput=True),
    }
    with tile.TileContext(nc) as tc:
        _emit(nc, tc, dram, c1, c2)
    nc.compile()
    return nc


def kernel(**inputs):
    global LAST_RESULTS
    q = np.asarray(inputs["query"], np.float32)
    k = np.asarray(inputs["key"], np.float32)
    v = np.asarray(inputs["value"], np.float32)
    msk = np.asarray(inputs["mask"], np.int32)
    ws = {nm: np.asarray(inputs["W" + nm], np.float32) for nm in "qkvo"}
    bs = {nm: np.asarray(inputs["b" + nm], np.float32) for nm in "qkvo"}
    alpha = float(1.0 / (1.0 + math.exp(-float(np.asarray(inputs["alpha_param"]).ravel()[0]))))
    c1 = alpha / math.sqrt(HD)
    c2 = 1.0 - alpha
    syn_bf = np.ascontiguousarray(
        np.asarray(inputs["syn_scores"])[:, :S, :S]).astype(ml_dtypes.bfloat16)
    boeff = (bs["v"].astype(np.float64) @ ws["o"].astype(np.float64)
             + bs["o"]).astype(np.float32)

    key_ = (round(c1, 12), round(c2, 12))
    if key_ not in _CACHE:
        _CACHE[key_] = _build(c1, c2)
    nc = _CACHE[key_]

    in_maps = []
    for b in range(B):
        in_maps.append({
            "xq": np.ascontiguousarray(q[b]),
            "xk": np.ascontiguousarray(k[b]),
            "xv": np.ascontiguousarray(v[b]),
            "msk": np.ascontiguousarray(msk[b]),
            "wq": ws["q"], "wk": ws["k"], "wv": ws["v"], "wo": ws["o"],
            "bq": bs["q"], "bk": bs["k"], "boeff": boeff,
            "syn": syn_bf,
        })

    kwargs = {}
    if TRACE:
        kwargs["trace"] = True
        if TRACE_TMPDIR:
            kwargs["tmpdir"] = TRACE_TMPDIR
    res = run_bass_kernel_spmd(nc, in_maps, core_ids=list(range(N_CORES)), **kwargs)
    LAST_RESULTS = res
    return np.stack([res.results[b]["out"] for b in range(B)], axis=0)
